# revision 31
# baseline (speedup 1.0000x reference)
"""CrossAttention (cosine-sim, learnable temperature) Trainium2 kernel.

Math (per batch element b, reference in fp32):
    qh  = (q @ Wq.T)   -> [Lq, C] -> heads [H, Lq, D]
    k,v = (kv @ Wkv.T) -> k,v [H, Lkv, D]
    qn = qh / (||qh||_d + eps); kn = k / (||k||_d + eps)
    attn = softmax(qn @ kn.T / tau); out = attn @ v
    y = out @ Wproj.T + bproj

Distribution: pure data-parallel over B=8 across the 8 NeuronCores (one
batch element per core, weights replicated, no collectives).

Device layout strategy: everything is kept "feature on partitions"
(transposed) so every matmul contraction dim lands on partitions:
    qT/kvT [C, L] (host pre-transposed), projections produce qnT/knT
    [C_out, L]; scores S^T [lkv, lq] = knT.T @ qnT per head; exp on ACT;
    P^T [lkv, lq] (bf16); out^T = [v | 1].T @ P^T gives both attn@v and
    the softmax sums (ones column); division by the sum and the final
    projection stay in the transposed domain; y [Lq, C] comes out in
    natural layout.

Normalization trick: rq = 1/(||qh||+eps) is applied to Q, and
rk/tau = 1/((||k||+eps)*tau) to K, before the scores matmul, so softmax
needs no further scaling.  Sum-of-squares over the head dim (on
partitions) is computed with a block-ones matmul; the per-row scales are
broadcast back across partitions with tiny K=2 / K=1 matmuls.
"""

import sys

sys.path.insert(0, "/opt/trn_rl_repo")

import numpy as np
import ml_dtypes

import concourse.bass as bass
import concourse.bacc as bacc
import concourse.mybir as mybir
from concourse.tile import TileContext
from concourse.bass_utils import run_bass_kernel_spmd

AF = mybir.ActivationFunctionType
ALU = mybir.AluOpType
F32 = mybir.dt.float32
F32R = mybir.dt.float32r
F16 = mybir.dt.float16
BF16 = mybir.dt.bfloat16

EPS = 1e-6
NCORES = 8


def r(ap):
    """fp32 AP -> float32r view (full-rate PE matmul on fp32 data)."""
    return ap.bitcast(F32R)


DEFAULT_KNOBS = dict(
    wt_bufs=2, sq_bufs=3, rbs_bufs=3, smalls_bufs=4,
    psA_bufs=4, psS_bufs=2, psB_bufs=2,
    pt_bufs=2, psSc_bufs=2, psPV_bufs=3, psBc_bufs=1,
    sbb_bufs=3, rsum_bufs=2, tmp_bufs=2, y_bufs=2,
    rb_evac="vector",         # rb broadcast psum -> SBUF evac engine
    sb_evac="vector",      # sb_b (1/sum broadcast) evac engine
    # NOTE: GPSIMD cannot access PSUM on TRN2 (BIR verifier) — psum
    # evacuations must go through ACT or DVE.
    norm_rsqrt=1,          # ACT Abs_reciprocal_sqrt for 1/(tau*||x||)
    recip_fast=1,          # custom-DVE reciprocal_approx_fast for 1/sum
    dma_order=1,           # priority-ordered input DMAs + per-ot W chunks
)


def build_nc(C=1024, H=16, LQ=1024, LKV=1024, knobs=None):
    kb = dict(DEFAULT_KNOBS)
    if knobs:
        kb.update(knobs)
    D = C // H          # head dim (64)
    P = 128
    OT = C // P         # feature tiles (8)
    CT = C // P         # contraction tiles (8)
    KT = LKV // P       # lkv partition tiles (8)
    HPT = P // D        # heads per 128-tile (2)
    CH = min(512, LQ)   # free-dim chunk per psum bank (fp32)
    NCH = LQ // CH      # chunks of Lq (2)
    VCH = min(512, C)   # chunk of output features for V projection
    NVCH = C // VCH

    nc = bacc.Bacc("TRN2", target_bir_lowering=False)

    qT = nc.dram_tensor("qT", [C, LQ], F16, kind="ExternalInput")
    kvT = nc.dram_tensor("kvT", [C, LKV], F16, kind="ExternalInput")
    wqT = nc.dram_tensor("wqT", [C, C], F16, kind="ExternalInput")
    wkT = nc.dram_tensor("wkT", [C, C], F16, kind="ExternalInput")
    wvT = nc.dram_tensor("wvT", [C, C], F16, kind="ExternalInput")
    wpT = nc.dram_tensor("wpT", [C, C], BF16, kind="ExternalInput")
    bproj = nc.dram_tensor("bproj", [1, C], BF16, kind="ExternalInput")
    tau_b = nc.dram_tensor("tau_b", [P, 1], F32, kind="ExternalInput")
    tau2_b = nc.dram_tensor("tau2_b", [P, 1], F32, kind="ExternalInput")
    ones_blk = nc.dram_tensor("ones_blk", [P, HPT], F16, kind="ExternalInput")
    blk2 = nc.dram_tensor("blk2", [HPT, P], F16, kind="ExternalInput")
    y = nc.dram_tensor("y", [LQ, C], F32, kind="ExternalOutput")

    qT_r = qT.rearrange("(ct p) l -> p ct l", p=P)
    kvT_r = kvT.rearrange("(ct p) l -> p ct l", p=P)
    wqT_r = wqT.rearrange("(ct p) o -> p ct o", p=P)
    wkT_r = wkT.rearrange("(ct p) o -> p ct o", p=P)
    wvT_r = wvT.rearrange("(ct p) o -> p ct o", p=P)
    wpT_r = wpT.rearrange("(ct p) o -> p ct o", p=P)
    y_r = y.rearrange("(yt p) o -> p yt o", p=P)

    with TileContext(nc) as tc:
        from contextlib import ExitStack

        with ExitStack() as stk:
            # ---------- persistent pools (live for the whole kernel) ----
            persist = stk.enter_context(tc.tile_pool(name="persist", bufs=1))
            qnT = persist.tile([P, OT, LQ], F16)      # qh * rq, transposed
            knT = persist.tile([P, OT, LKV], F16)     # k * rk/tau, transposed
            v_aug = persist.tile([P, KT, H, 2 * D], BF16)  # [ones|pad|v]
            oT = persist.tile([P, CT, LQ], BF16)       # (attn@v)/sum, transposed
            wp_sb = persist.tile([P, CT, C], BF16)
            consts = stk.enter_context(tc.tile_pool(name="consts", bufs=1))
            ones_blk_sb = consts.tile([P, HPT], F16)
            blk2_sb = consts.tile([HPT, P], F16)
            tau_sb = consts.tile([P, 1], F32)
            tau2_sb = consts.tile([P, 1], F32)
            ones64 = consts.tile([P, D], F32)
            ones64b = consts.tile([P, D], BF16)
            ones1 = consts.tile([1, P], BF16)
            bproj_sb = consts.tile([1, C], BF16)

            if not kb["dma_order"]:
                nc.sync.dma_start(out=ones_blk_sb, in_=ones_blk[:, :])
                nc.sync.dma_start(out=blk2_sb, in_=blk2[:, :])
                nc.sync.dma_start(out=tau_sb, in_=tau_b[:, :])
                nc.sync.dma_start(out=tau2_sb, in_=tau2_b[:, :])
            nc.vector.memset(ones64, 1.0)
            nc.vector.memset(ones64b, 1.0)
            nc.vector.memset(ones1, 1.0)
            # ones column FIRST: the softmax sum then lands on psum
            # partition 0, where the custom-DVE reciprocal is valid
            # (custom DVE ops silently no-op at partition offset != 0).
            nc.vector.memset(v_aug[:, :, :, 0:1], 1.0)
            nc.vector.memset(v_aug[:, :, :, 1:D], 0.0)
            if not kb["dma_order"]:
                nc.sync.dma_start(out=bproj_sb, in_=bproj[:, :])
                for ct in range(CT):
                    nc.sync.dma_start(out=wp_sb[:, ct, :], in_=wpT_r[:, ct, :])

            # ================= PHASE 1: projections =====================
            # (the whole body can be emitted `reps` times for benchmarking —
            # back-to-back repetitions in one NEFF isolate steady-state time)
            for _rep in range(kb.get("reps", 1)):
              with ExitStack() as repstk:
                # kvT + the V-projection weight stream live past phase 1 (the
                # second half of the V projection is interleaved into phase 2)
                p15 = repstk.enter_context(tc.tile_pool(name="p15", bufs=1))
                kvT_sb = p15.tile([P, CT, LKV], F16)
                wvp = repstk.enter_context(tc.tile_pool(name="wvp", bufs=1))

                if not kb["dma_order"]:
                    for ct in range(CT):
                        nc.sync.dma_start(out=kvT_sb[:, ct, :], in_=kvT_r[:, ct, :])

                hpc = VCH // D  # heads per v-projection chunk

                def emit_vproj(vch, vt, wv_t, pool, tag):
                    """One [128 lkv x VCH] tile of the V projection (natural
                    layout), written into the interleaved [v | ones] buffer."""
                    pv = pool.tile([P, VCH], F32, tag=tag)
                    for ct in range(CT):
                        nc.tensor.matmul(
                            pv,
                            kvT_sb[:, ct, vt * P : (vt + 1) * P],
                            wv_t[:, ct, :],
                            start=(ct == 0),
                            stop=(ct == CT - 1),
                        )
                    nc.vector.tensor_copy(
                        v_aug[:, vt, vch * hpc : (vch + 1) * hpc, D : 2 * D],
                        pv.rearrange("p (h d) -> p h d", d=D),
                    )

                with ExitStack() as p1:
                    ins = p1.enter_context(tc.tile_pool(name="ins", bufs=1))
                    qT_sb = ins.tile([P, CT, LQ], F16)
                    wst = p1.enter_context(tc.tile_pool(name="wst", bufs=2))
                    sqp = p1.enter_context(tc.tile_pool(name="sqp", bufs=kb["sq_bufs"]))
                    smalls = p1.enter_context(tc.tile_pool(name="smalls", bufs=kb["smalls_bufs"]))
                    rbs = p1.enter_context(tc.tile_pool(name="rbs", bufs=kb["rbs_bufs"]))
                    psA = p1.enter_context(
                        tc.tile_pool(name="psA", bufs=kb["psA_bufs"], space="PSUM")
                    )
                    psS = p1.enter_context(
                        tc.tile_pool(name="psS", bufs=kb["psS_bufs"], space="PSUM")
                    )
                    psB = p1.enter_context(
                        tc.tile_pool(name="psB", bufs=kb["psB_bufs"], space="PSUM")
                    )

                    wfull = {}
                    if kb["dma_order"]:
                        nc.sync.dma_start(out=qT_sb[:, :, :], in_=qT_r[:, :, :])
                    else:
                        for ct in range(CT):
                            nc.sync.dma_start(
                                out=qT_sb[:, ct, :], in_=qT_r[:, ct, :]
                            )
                    if kb["dma_order"] and kb.get("wfull", 1):
                        # priority order on the in-order DMA queue: wq in
                        # per-ot chunks (first matmul waits only chunk 0),
                        # then kvT, then wk chunks.
                        for side in range(2):
                            w_r = wqT_r if side == 0 else wkT_r
                            wf = wst.tile(
                                [P, CT, C], F16, tag=f"wf{side}", bufs=1,
                                name="wf",
                            )
                            wfull[side] = wf
                            for ot in range(OT):
                                sl = slice(ot * P, (ot + 1) * P)
                                nc.sync.dma_start(
                                    out=wf[:, :, sl], in_=w_r[:, :, sl]
                                )
                                if side == 0 and ot == 0:
                                    nc.sync.dma_start(
                                        out=ones_blk_sb, in_=ones_blk[:, :]
                                    )
                                    nc.sync.dma_start(
                                        out=blk2_sb, in_=blk2[:, :]
                                    )
                                    nc.sync.dma_start(
                                        out=tau_sb, in_=tau_b[:, :]
                                    )
                                    nc.sync.dma_start(
                                        out=tau2_sb, in_=tau2_b[:, :]
                                    )
                            if side == 0:
                                nc.sync.dma_start(
                                    out=kvT_sb[:, :, :], in_=kvT_r[:, :, :]
                                )

                    # --- software-pipelined projection+norm chunks ------------
                    # stage A (emit_mm):  proj matmuls -> ph psum; Square -> sq
                    # stage B (emit_ssq): block-ones matmul -> ssq; sqrt; +eps;
                    #                     reciprocal -> rrs
                    # stage C (emit_tail): broadcast matmul -> rb; evac; multiply
                    #                     -> qnT/knT (releases ph)
                    # Emission order interleaves stages two chunks apart so the
                    # in-order PE stream never waits on an ACT/DVE round-trip.
                    class Job:
                        pass

                    def stage_A(j):
                        j.ph = psA.tile([P, CH], F32, tag="ph", name="ph")
                        for ct in range(CT):
                            nc.tensor.matmul(
                                j.ph,
                                j.wt[:, ct, :],
                                j.x_sb[:, ct, j.sl],
                                start=(ct == 0),
                                stop=(ct == CT - 1),
                            )
                        j.sq = sqp.tile([P, CH], F16, tag="sq", name="sq")
                        nc.scalar.activation(j.sq, j.ph, AF.Square)

                    def stage_B(j):
                        j.ssq = psS.tile([HPT, CH], F32, tag="ssq", name="ssq")
                        nc.tensor.matmul(
                            j.ssq, ones_blk_sb, j.sq, start=True, stop=True
                        )
                        j.rrs = smalls.tile([HPT, CH], F16, tag="rrs", name="rrs")
                        if kb["norm_rsqrt"]:
                            # 1/(tau*||x||) = rsqrt(ssq * tau^2), single ACT op
                            # (eps dropped: ||x|| ~ 8 >> eps)
                            nc.scalar.activation(
                                j.rrs, j.ssq, AF.Abs_reciprocal_sqrt,
                                scale=tau2_sb[:HPT, :] if j.with_tau else 1.0,
                            )
                            return
                        rr = smalls.tile([HPT, CH], F32, tag="rr", name="rr")
                        nc.scalar.activation(rr, j.ssq, AF.Sqrt)
                        if j.with_tau:
                            nc.vector.tensor_scalar(
                                rr, rr, EPS, tau_sb[:HPT, :], op0=ALU.add,
                                op1=ALU.mult,
                            )
                        else:
                            nc.vector.tensor_scalar_add(rr, rr, EPS)
                        with nc.allow_low_precision(reason="fp16 inverse scale"):
                            nc.vector.reciprocal(j.rrs, rr)

                    def stage_C(j):
                        rb = psB.tile([P, CH], F32, tag="rb", name="rb")
                        nc.tensor.matmul(rb, blk2_sb, j.rrs, start=True, stop=True)
                        rb_sb = rbs.tile([P, CH], F32, tag="rb_sb", name="rb_sb")
                        if kb["rb_evac"] == "act":
                            nc.scalar.copy(rb_sb, rb)
                        elif kb["rb_evac"] == "gpsimd":
                            nc.gpsimd.tensor_copy(rb_sb, rb)
                        else:
                            nc.vector.tensor_copy(rb_sb, rb)
                        nc.vector.tensor_mul(j.out_t[:, j.ot, j.sl], j.ph, rb_sb)

                    jobs = []
                    if not kb["dma_order"] and kb.get("wfull", 1):
                        for side in range(2):
                            w_r = wqT_r if side == 0 else wkT_r
                            wf = wst.tile(
                                [P, CT, C], F16, tag=f"wf{side}", bufs=1,
                                name="wf",
                            )
                            for ct in range(CT):
                                nc.sync.dma_start(
                                    out=wf[:, ct, :], in_=w_r[:, ct, :]
                                )
                            wfull[side] = wf
                    if kb["dma_order"]:
                        # q-side jobs first: their inputs land first on the
                        # in-order DMA queue.
                        ot_side = [(ot, 0) for ot in range(OT)] + [
                            (ot, 1) for ot in range(OT)
                        ]
                    else:
                        ot_side = [
                            (ot, side) for ot in range(OT) for side in range(2)
                        ]
                    for ot, side in ot_side:
                            w_r = wqT_r if side == 0 else wkT_r
                            x_sb = qT_sb if side == 0 else kvT_sb
                            out_t = qnT if side == 0 else knT
                            L = LQ if side == 0 else LKV
                            if kb.get("wfull", 1):
                                wt = wfull[side][:, :, ot * P : (ot + 1) * P]
                            else:
                                wt = wst.tile(
                                    [P, CT, P], F16, tag="wt",
                                    bufs=kb["wt_bufs"], name="wt",
                                )
                                nc.sync.dma_start(
                                    out=wt,
                                    in_=w_r[:, :, ot * P : (ot + 1) * P],
                                )
                            for ch in range(L // CH):
                                j = Job()
                                j.wt, j.x_sb, j.out_t = wt, x_sb, out_t
                                j.ot, j.sl = ot, slice(ch * CH, (ch + 1) * CH)
                                j.with_tau = side == 1
                                jobs.append(j)

                    pd_b = kb.get("pd_b", 1)
                    pd_c = kb.get("pd_c", 2)
                    if kb.get("skip_square"):
                        def stage_A(j, _A=stage_A):
                            j.ph = psA.tile([P, CH], F32, tag="ph", name="ph")
                            for ct in range(CT):
                                nc.tensor.matmul(
                                    j.ph, j.wt[:, ct, :], j.x_sb[:, ct, j.sl],
                                    start=(ct == 0), stop=(ct == CT - 1),
                                )
                            j.sq = None
                        nc.vector.tensor_copy(qnT[:, 0, 0:CH], jobs[0].wt[:, 0, :].bitcast(F16)) if False else None
                    if kb.get("skip_tails"):
                        def stage_B(j):
                            pass
                        if kb.get("skip_evac"):
                            def stage_C(j):
                                pass
                        else:
                            def stage_C(j):
                                nc.scalar.activation(
                                    j.out_t[:, j.ot, j.sl], j.ph, AF.Copy
                                )
                    for i, j in enumerate(jobs):
                        stage_A(j)
                        if i >= pd_b:
                            stage_B(jobs[i - pd_b])
                        if i >= pd_c:
                            stage_C(jobs[i - pd_c])
                    for i in range(len(jobs) - pd_b, len(jobs)):
                        stage_B(jobs[i])
                        if i - pd_c + pd_b >= 0 and i - pd_c + pd_b < len(jobs) and i - pd_c + pd_b >= len(jobs) - pd_c:
                            pass
                    for i in range(len(jobs) - pd_c, len(jobs)):
                        stage_C(jobs[i])

                    # first half of the V projection (heads 0..hpc-1)
                    if not kb.get("skip_vproj0"):
                        wv_t = wvp.tile([P, CT, VCH], F16, tag="wv", bufs=1, name="wv")
                        nc.sync.dma_start(out=wv_t, in_=wvT_r[:, :, 0:VCH])
                        for vt in range(KT):
                            emit_vproj(0, vt, wv_t, psA, "ph")

                # ================= PHASE 2: attention per head ==============
                with ExitStack() as p2:
                    ptp = p2.enter_context(tc.tile_pool(name="ptp", bufs=kb["pt_bufs"]))
                    rsp = p2.enter_context(tc.tile_pool(name="rsp", bufs=kb["rsum_bufs"]))
                    sbb = p2.enter_context(tc.tile_pool(name="sbb", bufs=kb["sbb_bufs"]))
                    tmpp = p2.enter_context(tc.tile_pool(name="tmpp", bufs=kb["tmp_bufs"]))
                    yp = p2.enter_context(tc.tile_pool(name="yp", bufs=kb["y_bufs"]))
                    ymp = p2.enter_context(tc.tile_pool(name="ymp", bufs=1))
                    psSc = p2.enter_context(
                        tc.tile_pool(name="psSc", bufs=kb["psSc_bufs"], space="PSUM")
                    )
                    psPV = p2.enter_context(
                        tc.tile_pool(name="psPV", bufs=kb["psPV_bufs"], space="PSUM")
                    )
                    psBc = p2.enter_context(
                        tc.tile_pool(name="psBc", bufs=kb["psBc_bufs"], space="PSUM")
                    )

                    if kb["dma_order"]:
                        # wp/bproj are first needed by phase 2/3 — loading
                        # them here keeps the phase-1 critical DMAs in front.
                        nc.sync.dma_start(out=bproj_sb, in_=bproj[:, :])
                        nc.sync.dma_start(out=wp_sb[:, :, :], in_=wpT_r[:, :, :])

                    if kb.get("only_phase1"):
                        heads = []
                    else:
                        heads = list(range(H))

                    # V projection, second half: interleaved into the head loop
                    # (fills the PE gap while it waits for the sum reciprocal).
                    if heads and NVCH > 1:
                        wv2 = wvp.tile(
                            [P, CT, VCH], F16, tag="wv", bufs=1, name="wv2"
                        )
                        nc.sync.dma_start(out=wv2, in_=wvT_r[:, :, VCH : 2 * VCH])

                    def emit_scores(h):
                        par, ot = h % HPT, h // HPT
                        rows = slice(par * D, (par + 1) * D)
                        pt = ptp.tile([P, KT, LQ], BF16, tag="pt", name="pt")
                        for kt in range(KT):
                            ps_s = psSc.tile([P, LQ], F32, tag="ps_s", name="ps_s")
                            for ch in range(NCH):
                                sl = slice(ch * CH, (ch + 1) * CH)
                                nc.tensor.matmul(
                                    ps_s[:, sl],
                                    knT[rows, ot, kt * P : (kt + 1) * P],
                                    qnT[rows, ot, sl],
                                    start=True,
                                    stop=True,
                                )
                            nc.scalar.activation(pt[:, kt, :], ps_s, AF.Exp)
                        return pt

                    def emit_pv(h, pt):
                        rsum = rsp.tile([P, LQ], BF16, tag="rsum", name="rsum")
                        pvs = []
                        for ch in range(NCH):
                            sl = slice(ch * CH, (ch + 1) * CH)
                            pv = psPV.tile(
                                [2 * D, CH], F32, tag="ps_pv", name="ps_pv"
                            )
                            pvs.append(pv)
                            for kt in range(KT):
                                nc.tensor.matmul(
                                    pv,
                                    v_aug[:, kt, h, :],
                                    pt[:, kt, sl],
                                    start=(kt == 0),
                                    stop=(kt == KT - 1),
                                )
                            if kb["recip_fast"]:
                                # ~51-ULP custom-DVE reciprocal, ~5x faster
                                # than the iterative DVE reciprocal; bf16
                                # output cast happens on the DVE write port
                                from concourse.dve_ops import (
                                    RECIP_APPROX_FAST_CONSTS as _RC,
                                    RECIPROCAL_APPROX_FAST as _RF,
                                )
                                nc.vector._custom_dve(
                                    _RF,
                                    out=rsum[0:1, sl],
                                    in0=pv[0:1, :],
                                    s0=_RC["s0"], s1=_RC["s1"],
                                    imm2=_RC["imm2"],
                                )
                            else:
                                with nc.allow_low_precision(
                                    reason="bf16 softmax sum"
                                ):
                                    nc.vector.reciprocal(
                                        rsum[0:1, sl], pv[0:1, :]
                                    )
                        return pvs, rsum

                    def emit_tail(h, pvs, rsum):
                        par, ot = h % HPT, h // HPT
                        rows = slice(par * D, (par + 1) * D)
                        for ch in range(NCH):
                            sl = slice(ch * CH, (ch + 1) * CH)
                            ps_b = psBc.tile([D, CH], F32, tag="ps_b", name="ps_b")
                            nc.tensor.matmul(
                                ps_b,
                                ones64b[0:1, :],
                                rsum[0:1, sl],
                                start=True,
                                stop=True,
                            )
                            sb_b = sbb.tile([D, CH], F32, tag="sb_b", name="sb_b")
                            if kb["sb_evac"] == "gpsimd":
                                nc.gpsimd.tensor_copy(sb_b, ps_b)
                            else:
                                nc.vector.tensor_copy(sb_b, ps_b)
                            nc.vector.tensor_mul(
                                oT[rows, ot, sl], pvs[ch][D : 2 * D, :], sb_b
                            )

                    bias_bc = None
                    if heads and not (
                        bool(kb.get("split_out", 1))
                        and H >= 16
                        and not (bool(kb.get("pair", 1)) and HPT == 2)
                    ):
                        # broadcast bproj across partitions once; phase 3
                        # then folds the bias into the psum evac (tensor_add)
                        # instead of 2 extra matmuls per yt tile
                        ps_bb = psSc.tile([P, LQ], F32, tag="ps_s", name="ps_bb")
                        for vch in range(NVCH):
                            sl = slice(vch * VCH, (vch + 1) * VCH)
                            nc.tensor.matmul(
                                ps_bb[:, sl], ones1, bproj_sb[:, sl],
                                start=True, stop=True,
                            )
                        bias_bc = ymp.tile([P, C], BF16, name="bias_bc")
                        nc.vector.tensor_copy(bias_bc, ps_bb[:, 0:C])

                    use_pair = bool(kb.get("pair", 1)) and HPT == 2 and heads
                    # pair mode needs 4 pt buffers; drop y_mid to fit SBUF
                    split_out = (
                        bool(kb.get("split_out", 1)) and H >= 16 and not use_pair
                    )
                    ptb = 4 if use_pair else None
                    y_mid = None
                    if split_out:
                        y_mid = ymp.tile([P, LQ // P, C], BF16, name="y_mid")

                    def emit_out_half1(u):
                        # u indexes (yt, vch) units; contraction tiles ct<CT/2
                        yt, vch = divmod(u, NVCH)
                        sl = slice(vch * VCH, (vch + 1) * VCH)
                        ps_h = psPV.tile(
                            [P, VCH], F32, tag="ps_pv", name="ps_h"
                        )
                        for ct in range(CT // 2):
                            nc.tensor.matmul(
                                ps_h,
                                oT[:, ct, yt * P : (yt + 1) * P],
                                wp_sb[:, ct, sl],
                                start=(ct == 0),
                                stop=(ct == CT // 2 - 1),
                            )
                        nc.vector.tensor_copy(y_mid[:, yt, sl], ps_h)

                    def emit_scores_pair(h0, h1):
                        """Scores+exp for an even/odd head pair. The two
                        heads' matmuls are interleaved: they sit on PE row
                        groups 0-1 and 2-3 (base partitions 0 and 64), so
                        adjacent matmuls execute concurrently on hardware."""
                        ot = h0 // HPT
                        r0 = slice(0, D)
                        r1 = slice(D, 2 * D)
                        pt0 = ptp.tile(
                            [P, KT, LQ], BF16, tag="pt", name="pt0", bufs=ptb
                        )
                        pt1 = ptp.tile(
                            [P, KT, LQ], BF16, tag="pt", name="pt1", bufs=ptb
                        )
                        for kt in range(KT):
                            kl = slice(kt * P, (kt + 1) * P)
                            s0 = psSc.tile([P, LQ], F32, tag="ps_s", name="s0")
                            s1 = psSc.tile([P, LQ], F32, tag="ps_s", name="s1")
                            for ch in range(NCH):
                                sl = slice(ch * CH, (ch + 1) * CH)
                                nc.tensor.matmul(
                                    s0[:, sl], knT[r0, ot, kl],
                                    qnT[r0, ot, sl], start=True, stop=True,
                                )
                                nc.tensor.matmul(
                                    s1[:, sl], knT[r1, ot, kl],
                                    qnT[r1, ot, sl], start=True, stop=True,
                                )
                            nc.scalar.activation(pt0[:, kt, :], s0, AF.Exp)
                            nc.scalar.activation(pt1[:, kt, :], s1, AF.Exp)
                        return pt0, pt1

                    nunits = (LQ // P) * NVCH
                    emitted_units = 0
                    if use_pair:
                        def process_pair(pr, pts):
                            for i, hp in enumerate(pr):
                                pvs, rsum = emit_pv(hp, pts[i])
                                if NVCH > 1 and hp < KT:
                                    emit_vproj(1, hp, wv2, psPV, "ps_pv")
                                emit_tail(hp, pvs, rsum)

                        pend = None
                        for pi in range(len(heads) // 2):
                            pr = (heads[2 * pi], heads[2 * pi + 1])
                            pts = emit_scores_pair(*pr)
                            if pend is not None:
                                process_pair(*pend)
                            pend = (pr, pts)
                        if pend is not None:
                            process_pair(*pend)
                    else:
                        pend = None
                        for h in heads:
                            pt = emit_scores(h)
                            if pend is not None:
                                hp, ptp_ = pend
                                pvs, rsum = emit_pv(hp, ptp_)
                                if NVCH > 1 and hp < KT:
                                    emit_vproj(1, hp, wv2, psPV, "ps_pv")
                                emit_tail(hp, pvs, rsum)
                                if split_out and hp >= H - KT:
                                    u0 = (hp - (H - KT)) * 2
                                    for u in range(u0, min(u0 + 2, nunits)):
                                        emit_out_half1(u)
                                        emitted_units = max(
                                            emitted_units, u + 1
                                        )
                            pend = (h, pt)
                        if pend is not None:
                            hp, ptp_ = pend
                            pvs, rsum = emit_pv(hp, ptp_)
                            emit_tail(hp, pvs, rsum)
                    if split_out:
                        for u in range(emitted_units, nunits):
                            emit_out_half1(u)

                    # ============ PHASE 3: output projection ================
                    ct0 = CT // 2 if split_out else 0
                    for yt in ([] if kb.get("only_phase1") else range(LQ // P)):
                        ps_y = psSc.tile([P, C], F32, tag="ps_s", name="ps_y")
                        for vch in range(NVCH):
                            sl = slice(vch * VCH, (vch + 1) * VCH)
                            for ct in range(ct0, CT):
                                nc.tensor.matmul(
                                    ps_y[:, sl],
                                    oT[:, ct, yt * P : (yt + 1) * P],
                                    wp_sb[:, ct, sl],
                                    start=(ct == ct0),
                                    stop=(bias_bc is not None and ct == CT - 1),
                                )
                            if bias_bc is None:
                                nc.tensor.matmul(
                                    ps_y[:, sl],
                                    ones1,
                                    bproj_sb[:, sl],
                                    start=False,
                                    stop=True,
                                )
                        y_sb = yp.tile([P, C], F32, tag="y_sb", name="y_sb")
                        if split_out:
                            nc.vector.tensor_add(y_sb, ps_y, y_mid[:, yt, :])
                        elif bias_bc is not None:
                            nc.vector.tensor_add(y_sb, ps_y, bias_bc)
                        else:
                            nc.vector.tensor_copy(y_sb, ps_y)
                        nc.sync.dma_start(out=y_r[:, yt, :], in_=y_sb)

    nc.finalize()
    return nc


_NC_CACHE = {}


def _get_nc(C, H, LQ, LKV):
    key = (C, H, LQ, LKV)
    if key not in _NC_CACHE:
        _NC_CACHE[key] = build_nc(C, H, LQ, LKV)
    return _NC_CACHE[key]


def _host_inputs(q, kv, Wq, Wkv, Wproj, bproj, tau, H):
    B, LQ, C = q.shape
    LKV = kv.shape[1]
    P, D = 128, C // H
    HPT = P // D

    f16 = lambda a: np.ascontiguousarray(np.asarray(a, dtype=np.float32).astype(np.float16))
    bf16 = lambda a: np.ascontiguousarray(
        np.asarray(a, dtype=np.float32).astype(ml_dtypes.bfloat16)
    )

    wqT = f16(np.asarray(Wq).T)
    wkT = f16(np.asarray(Wkv)[:C].T)
    wvT = f16(np.asarray(Wkv)[C:].T)
    wpT = bf16(np.asarray(Wproj).T)
    bp = bf16(np.asarray(bproj).reshape(1, C))
    tau_b = np.full((P, 1), float(np.asarray(tau)), dtype=np.float32)
    tau2_b = np.full((P, 1), float(np.asarray(tau)) ** 2, dtype=np.float32)
    ones_blk = np.zeros((P, HPT), dtype=np.float16)
    for p in range(P):
        ones_blk[p, p // D] = 1.0
    blk2 = np.ascontiguousarray(ones_blk.T)

    shared = {
        "wqT": wqT, "wkT": wkT, "wvT": wvT, "wpT": wpT, "bproj": bp,
        "tau_b": tau_b, "tau2_b": tau2_b, "ones_blk": ones_blk, "blk2": blk2,
    }
    qn = np.asarray(q, dtype=np.float32)
    kvn = np.asarray(kv, dtype=np.float32)
    in_maps = []
    for b in range(B):
        m = dict(shared)
        m["qT"] = f16(qn[b].T)
        m["kvT"] = f16(kvn[b].T)
        in_maps.append(m)
    return in_maps


def kernel(q, kv, Wq, Wkv, Wproj, bproj, tau, _trace=False):
    B, LQ, C = q.shape
    LKV = kv.shape[1]
    H = 16 if C == 1024 else max(1, C // 64)
    assert B == NCORES, f"expected B == {NCORES}, got {B}"

    nc = _get_nc(C, H, LQ, LKV)
    in_maps = _host_inputs(q, kv, Wq, Wkv, Wproj, bproj, tau, H)
    res = run_bass_kernel_spmd(
        nc, in_maps, core_ids=list(range(NCORES)), trace=_trace
    )
    out = np.stack([res.results[b]["y"] for b in range(B)], axis=0)
    out = out.astype(np.asarray(q).dtype)
    if _trace:
        kernel._last_result = res
    return out



# revision 33
# speedup vs baseline: 1.0772x; 1.0772x over previous
"""CrossAttention (cosine-sim, learnable temperature) Trainium2 kernel.

Math (per batch element b, reference in fp32):
    qh  = (q @ Wq.T)   -> [Lq, C] -> heads [H, Lq, D]
    k,v = (kv @ Wkv.T) -> k,v [H, Lkv, D]
    qn = qh / (||qh||_d + eps); kn = k / (||k||_d + eps)
    attn = softmax(qn @ kn.T / tau); out = attn @ v
    y = out @ Wproj.T + bproj

Distribution: pure data-parallel over B=8 across the 8 NeuronCores (one
batch element per core, weights replicated, no collectives).

Device layout strategy: everything is kept "feature on partitions"
(transposed) so every matmul contraction dim lands on partitions:
    qT/kvT [C, L] (host pre-transposed), projections produce qnT/knT
    [C_out, L]; scores S^T [lkv, lq] = knT.T @ qnT per head; exp on ACT;
    P^T [lkv, lq] (bf16); out^T = [v | 1].T @ P^T gives both attn@v and
    the softmax sums (ones column); division by the sum and the final
    projection stay in the transposed domain; y [Lq, C] comes out in
    natural layout.

Normalization trick: rq = 1/(||qh||+eps) is applied to Q, and
rk/tau = 1/((||k||+eps)*tau) to K, before the scores matmul, so softmax
needs no further scaling.  Sum-of-squares over the head dim (on
partitions) is computed with a block-ones matmul; the per-row scales are
broadcast back across partitions with tiny K=2 / K=1 matmuls.
"""

import sys

sys.path.insert(0, "/opt/trn_rl_repo")

import numpy as np
import ml_dtypes

import concourse.bass as bass
import concourse.bacc as bacc
import concourse.mybir as mybir
from concourse.tile import TileContext
from concourse.bass_utils import run_bass_kernel_spmd

AF = mybir.ActivationFunctionType
ALU = mybir.AluOpType
F32 = mybir.dt.float32
F32R = mybir.dt.float32r
F16 = mybir.dt.float16
BF16 = mybir.dt.bfloat16

EPS = 1e-6
NCORES = 8


def r(ap):
    """fp32 AP -> float32r view (full-rate PE matmul on fp32 data)."""
    return ap.bitcast(F32R)


DEFAULT_KNOBS = dict(
    wt_bufs=2, sq_bufs=3, rbs_bufs=3, smalls_bufs=4,
    psA_bufs=4, psS_bufs=2, psB_bufs=2,
    pt_bufs=2, psSc_bufs=2, psPV_bufs=3, psBc_bufs=1,
    sbb_bufs=3, rsum_bufs=2, tmp_bufs=2, y_bufs=2,
    rb_evac="vector",         # rb broadcast psum -> SBUF evac engine
    sb_evac="vector",      # sb_b (1/sum broadcast) evac engine
    # NOTE: GPSIMD cannot access PSUM on TRN2 (BIR verifier) — psum
    # evacuations must go through ACT or DVE.
    norm_rsqrt=1,          # ACT Abs_reciprocal_sqrt for 1/(tau*||x||)
    recip_fast=1,          # custom-DVE reciprocal_approx_fast for 1/sum
    dma_order=1,           # priority-ordered input DMAs + per-ot W chunks
)


def build_nc(C=1024, H=16, LQ=1024, LKV=1024, knobs=None):
    kb = dict(DEFAULT_KNOBS)
    if knobs:
        kb.update(knobs)
    D = C // H          # head dim (64)
    P = 128
    OT = C // P         # feature tiles (8)
    CT = C // P         # contraction tiles (8)
    KT = LKV // P       # lkv partition tiles (8)
    HPT = P // D        # heads per 128-tile (2)
    CH = min(512, LQ)   # free-dim chunk per psum bank (fp32)
    NCH = LQ // CH      # chunks of Lq (2)
    VCH = min(512, C)   # chunk of output features for V projection
    NVCH = C // VCH

    nc = bacc.Bacc("TRN2", target_bir_lowering=False)

    qT = nc.dram_tensor("qT", [C, LQ], F16, kind="ExternalInput")
    kvT = nc.dram_tensor("kvT", [C, LKV], F16, kind="ExternalInput")
    wqT = nc.dram_tensor("wqT", [C, C], F16, kind="ExternalInput")
    wkT = nc.dram_tensor("wkT", [C, C], F16, kind="ExternalInput")
    wvT = nc.dram_tensor("wvT", [C, C], F16, kind="ExternalInput")
    wpT = nc.dram_tensor("wpT", [C, C], BF16, kind="ExternalInput")
    bproj = nc.dram_tensor("bproj", [1, C], BF16, kind="ExternalInput")
    tau_b = nc.dram_tensor("tau_b", [P, 1], F32, kind="ExternalInput")
    tau2_b = nc.dram_tensor("tau2_b", [P, 1], F32, kind="ExternalInput")
    ones_blk = nc.dram_tensor("ones_blk", [P, HPT], F16, kind="ExternalInput")
    blk2 = nc.dram_tensor("blk2", [HPT, P], F16, kind="ExternalInput")
    y = nc.dram_tensor("y", [LQ, C], F32, kind="ExternalOutput")

    qT_r = qT.rearrange("(ct p) l -> p ct l", p=P)
    kvT_r = kvT.rearrange("(ct p) l -> p ct l", p=P)
    wqT_r = wqT.rearrange("(ct p) o -> p ct o", p=P)
    wkT_r = wkT.rearrange("(ct p) o -> p ct o", p=P)
    wvT_r = wvT.rearrange("(ct p) o -> p ct o", p=P)
    wpT_r = wpT.rearrange("(ct p) o -> p ct o", p=P)
    y_r = y.rearrange("(yt p) o -> p yt o", p=P)

    with TileContext(nc) as tc:
        from contextlib import ExitStack

        with ExitStack() as stk:
            # ---------- persistent pools (live for the whole kernel) ----
            persist = stk.enter_context(tc.tile_pool(name="persist", bufs=1))
            qnT = persist.tile([P, OT, LQ], F16)      # qh * rq, transposed
            knT = persist.tile([P, OT, LKV], F16)     # k * rk/tau, transposed
            v_aug = persist.tile([P, KT, H, 2 * D], BF16)  # [ones|pad|v]
            oT = persist.tile([P, CT, LQ], BF16)       # (attn@v)/sum, transposed
            wp_sb = persist.tile([P, CT, C], BF16)
            consts = stk.enter_context(tc.tile_pool(name="consts", bufs=1))
            ones_blk_sb = consts.tile([P, HPT], F16)
            blk2_sb = consts.tile([HPT, P], F16)
            tau_sb = consts.tile([P, 1], F32)
            tau2_sb = consts.tile([P, 1], F32)
            ones64 = consts.tile([P, D], F32)
            ones64b = consts.tile([P, D], BF16)
            ones1 = consts.tile([1, P], BF16)
            bproj_sb = consts.tile([1, C], BF16)

            if not kb["dma_order"]:
                nc.sync.dma_start(out=ones_blk_sb, in_=ones_blk[:, :])
                nc.sync.dma_start(out=blk2_sb, in_=blk2[:, :])
                nc.sync.dma_start(out=tau_sb, in_=tau_b[:, :])
                nc.sync.dma_start(out=tau2_sb, in_=tau2_b[:, :])
            nc.vector.memset(ones64, 1.0)
            nc.vector.memset(ones64b, 1.0)
            nc.vector.memset(ones1, 1.0)
            # ones column FIRST: the softmax sum then lands on psum
            # partition 0, where the custom-DVE reciprocal is valid
            # (custom DVE ops silently no-op at partition offset != 0).
            nc.vector.memset(v_aug[:, :, :, 0:1], 1.0)
            nc.vector.memset(v_aug[:, :, :, 1:D], 0.0)
            if not kb["dma_order"]:
                nc.sync.dma_start(out=bproj_sb, in_=bproj[:, :])
                for ct in range(CT):
                    nc.sync.dma_start(out=wp_sb[:, ct, :], in_=wpT_r[:, ct, :])

            # ================= PHASE 1: projections =====================
            # (the whole body can be emitted `reps` times for benchmarking —
            # back-to-back repetitions in one NEFF isolate steady-state time)
            for _rep in range(kb.get("reps", 1)):
              with ExitStack() as repstk:
                # kvT + the V-projection weight stream live past phase 1 (the
                # second half of the V projection is interleaved into phase 2)
                p15 = repstk.enter_context(tc.tile_pool(name="p15", bufs=1))
                kvT_sb = p15.tile([P, CT, LKV], F16)
                wvp = repstk.enter_context(tc.tile_pool(name="wvp", bufs=1))

                if not kb["dma_order"]:
                    for ct in range(CT):
                        nc.sync.dma_start(out=kvT_sb[:, ct, :], in_=kvT_r[:, ct, :])

                hpc = VCH // D  # heads per v-projection chunk

                def emit_vproj(vch, vt, wv_t, pool, tag):
                    """One [128 lkv x VCH] tile of the V projection (natural
                    layout), written into the interleaved [v | ones] buffer."""
                    pv = pool.tile([P, VCH], F32, tag=tag)
                    for ct in range(CT):
                        nc.tensor.matmul(
                            pv,
                            kvT_sb[:, ct, vt * P : (vt + 1) * P],
                            wv_t[:, ct, :],
                            start=(ct == 0),
                            stop=(ct == CT - 1),
                        )
                    nc.vector.tensor_copy(
                        v_aug[:, vt, vch * hpc : (vch + 1) * hpc, D : 2 * D],
                        pv.rearrange("p (h d) -> p h d", d=D),
                    )

                with ExitStack() as p1:
                    ins = p1.enter_context(tc.tile_pool(name="ins", bufs=1))
                    qT_sb = ins.tile([P, CT, LQ], F16)
                    wst = p1.enter_context(tc.tile_pool(name="wst", bufs=2))
                    sqp = p1.enter_context(tc.tile_pool(name="sqp", bufs=kb["sq_bufs"]))
                    smalls = p1.enter_context(tc.tile_pool(name="smalls", bufs=kb["smalls_bufs"]))
                    rbs = p1.enter_context(tc.tile_pool(name="rbs", bufs=kb["rbs_bufs"]))
                    psA = p1.enter_context(
                        tc.tile_pool(name="psA", bufs=kb["psA_bufs"], space="PSUM")
                    )
                    psS = p1.enter_context(
                        tc.tile_pool(name="psS", bufs=kb["psS_bufs"], space="PSUM")
                    )
                    psB = p1.enter_context(
                        tc.tile_pool(name="psB", bufs=kb["psB_bufs"], space="PSUM")
                    )

                    wfull = {}
                    if kb["dma_order"]:
                        nc.sync.dma_start(
                            out=qT_sb[:, :, 0:CH], in_=qT_r[:, :, 0:CH]
                        )
                    else:
                        for ct in range(CT):
                            nc.sync.dma_start(
                                out=qT_sb[:, ct, :], in_=qT_r[:, ct, :]
                            )
                    if kb["dma_order"] and kb.get("wfull", 1):
                        # priority order on the in-order DMA queue: wq in
                        # per-ot chunks (first matmul waits only chunk 0),
                        # then kvT, then wk chunks.
                        for side in range(2):
                            w_r = wqT_r if side == 0 else wkT_r
                            wf = wst.tile(
                                [P, CT, C], F16, tag=f"wf{side}", bufs=1,
                                name="wf",
                            )
                            wfull[side] = wf
                            for ot in range(OT):
                                sl = slice(ot * P, (ot + 1) * P)
                                nc.sync.dma_start(
                                    out=wf[:, :, sl], in_=w_r[:, :, sl]
                                )
                                if side == 0 and ot == 0:
                                    nc.sync.dma_start(
                                        out=qT_sb[:, :, CH:LQ],
                                        in_=qT_r[:, :, CH:LQ],
                                    )
                                    nc.sync.dma_start(
                                        out=ones_blk_sb, in_=ones_blk[:, :]
                                    )
                                    nc.sync.dma_start(
                                        out=blk2_sb, in_=blk2[:, :]
                                    )
                                    nc.sync.dma_start(
                                        out=tau_sb, in_=tau_b[:, :]
                                    )
                                    nc.sync.dma_start(
                                        out=tau2_sb, in_=tau2_b[:, :]
                                    )
                            if side == 0:
                                nc.sync.dma_start(
                                    out=kvT_sb[:, :, :], in_=kvT_r[:, :, :]
                                )

                    # --- software-pipelined projection+norm chunks ------------
                    # stage A (emit_mm):  proj matmuls -> ph psum; Square -> sq
                    # stage B (emit_ssq): block-ones matmul -> ssq; sqrt; +eps;
                    #                     reciprocal -> rrs
                    # stage C (emit_tail): broadcast matmul -> rb; evac; multiply
                    #                     -> qnT/knT (releases ph)
                    # Emission order interleaves stages two chunks apart so the
                    # in-order PE stream never waits on an ACT/DVE round-trip.
                    class Job:
                        pass

                    def stage_A(j):
                        j.ph = psA.tile([P, CH], F32, tag="ph", name="ph")
                        for ct in range(CT):
                            nc.tensor.matmul(
                                j.ph,
                                j.wt[:, ct, :],
                                j.x_sb[:, ct, j.sl],
                                start=(ct == 0),
                                stop=(ct == CT - 1),
                            )
                        j.sq = sqp.tile([P, CH], F16, tag="sq", name="sq")
                        nc.scalar.activation(j.sq, j.ph, AF.Square)

                    def stage_B(j):
                        j.ssq = psS.tile([HPT, CH], F32, tag="ssq", name="ssq")
                        nc.tensor.matmul(
                            j.ssq, ones_blk_sb, j.sq, start=True, stop=True
                        )
                        j.rrs = smalls.tile([HPT, CH], F16, tag="rrs", name="rrs")
                        if kb["norm_rsqrt"]:
                            # 1/(tau*||x||) = rsqrt(ssq * tau^2), single ACT op
                            # (eps dropped: ||x|| ~ 8 >> eps)
                            nc.scalar.activation(
                                j.rrs, j.ssq, AF.Abs_reciprocal_sqrt,
                                scale=tau2_sb[:HPT, :] if j.with_tau else 1.0,
                            )
                            return
                        rr = smalls.tile([HPT, CH], F32, tag="rr", name="rr")
                        nc.scalar.activation(rr, j.ssq, AF.Sqrt)
                        if j.with_tau:
                            nc.vector.tensor_scalar(
                                rr, rr, EPS, tau_sb[:HPT, :], op0=ALU.add,
                                op1=ALU.mult,
                            )
                        else:
                            nc.vector.tensor_scalar_add(rr, rr, EPS)
                        with nc.allow_low_precision(reason="fp16 inverse scale"):
                            nc.vector.reciprocal(j.rrs, rr)

                    def stage_C(j):
                        rb = psB.tile([P, CH], F32, tag="rb", name="rb")
                        nc.tensor.matmul(rb, blk2_sb, j.rrs, start=True, stop=True)
                        rb_sb = rbs.tile([P, CH], F32, tag="rb_sb", name="rb_sb")
                        if kb["rb_evac"] == "act":
                            nc.scalar.copy(rb_sb, rb)
                        elif kb["rb_evac"] == "gpsimd":
                            nc.gpsimd.tensor_copy(rb_sb, rb)
                        else:
                            nc.vector.tensor_copy(rb_sb, rb)
                        nc.vector.tensor_mul(j.out_t[:, j.ot, j.sl], j.ph, rb_sb)

                    jobs = []
                    if not kb["dma_order"] and kb.get("wfull", 1):
                        for side in range(2):
                            w_r = wqT_r if side == 0 else wkT_r
                            wf = wst.tile(
                                [P, CT, C], F16, tag=f"wf{side}", bufs=1,
                                name="wf",
                            )
                            for ct in range(CT):
                                nc.sync.dma_start(
                                    out=wf[:, ct, :], in_=w_r[:, ct, :]
                                )
                            wfull[side] = wf
                    if kb["dma_order"]:
                        # q-side jobs first: their inputs land first on the
                        # in-order DMA queue.
                        ot_side = [(ot, 0) for ot in range(OT)] + [
                            (ot, 1) for ot in range(OT)
                        ]
                    else:
                        ot_side = [
                            (ot, side) for ot in range(OT) for side in range(2)
                        ]
                    for ot, side in ot_side:
                            w_r = wqT_r if side == 0 else wkT_r
                            x_sb = qT_sb if side == 0 else kvT_sb
                            out_t = qnT if side == 0 else knT
                            L = LQ if side == 0 else LKV
                            if kb.get("wfull", 1):
                                wt = wfull[side][:, :, ot * P : (ot + 1) * P]
                            else:
                                wt = wst.tile(
                                    [P, CT, P], F16, tag="wt",
                                    bufs=kb["wt_bufs"], name="wt",
                                )
                                nc.sync.dma_start(
                                    out=wt,
                                    in_=w_r[:, :, ot * P : (ot + 1) * P],
                                )
                            for ch in range(L // CH):
                                j = Job()
                                j.wt, j.x_sb, j.out_t = wt, x_sb, out_t
                                j.ot, j.sl = ot, slice(ch * CH, (ch + 1) * CH)
                                j.with_tau = side == 1
                                jobs.append(j)

                    pd_b = kb.get("pd_b", 1)
                    pd_c = kb.get("pd_c", 2)
                    if kb.get("skip_square"):
                        def stage_A(j, _A=stage_A):
                            j.ph = psA.tile([P, CH], F32, tag="ph", name="ph")
                            for ct in range(CT):
                                nc.tensor.matmul(
                                    j.ph, j.wt[:, ct, :], j.x_sb[:, ct, j.sl],
                                    start=(ct == 0), stop=(ct == CT - 1),
                                )
                            j.sq = None
                        nc.vector.tensor_copy(qnT[:, 0, 0:CH], jobs[0].wt[:, 0, :].bitcast(F16)) if False else None
                    if kb.get("skip_tails"):
                        def stage_B(j):
                            pass
                        if kb.get("skip_evac"):
                            def stage_C(j):
                                pass
                        else:
                            def stage_C(j):
                                nc.scalar.activation(
                                    j.out_t[:, j.ot, j.sl], j.ph, AF.Copy
                                )
                    for i, j in enumerate(jobs):
                        stage_A(j)
                        if i >= pd_b:
                            stage_B(jobs[i - pd_b])
                        if i >= pd_c:
                            stage_C(jobs[i - pd_c])
                    for i in range(len(jobs) - pd_b, len(jobs)):
                        stage_B(jobs[i])
                        if i - pd_c + pd_b >= 0 and i - pd_c + pd_b < len(jobs) and i - pd_c + pd_b >= len(jobs) - pd_c:
                            pass
                    for i in range(len(jobs) - pd_c, len(jobs)):
                        stage_C(jobs[i])

                    # first half of the V projection (heads 0..hpc-1)
                    if not kb.get("skip_vproj0"):
                        wv_t = wvp.tile([P, CT, VCH], F16, tag="wv", bufs=1, name="wv")
                        nc.sync.dma_start(out=wv_t, in_=wvT_r[:, :, 0:VCH])
                        for vt in range(KT):
                            emit_vproj(0, vt, wv_t, psA, "ph")

                # ================= PHASE 2: attention per head ==============
                with ExitStack() as p2:
                    ptp = p2.enter_context(tc.tile_pool(name="ptp", bufs=kb["pt_bufs"]))
                    rsp = p2.enter_context(tc.tile_pool(name="rsp", bufs=kb["rsum_bufs"]))
                    sbb = p2.enter_context(tc.tile_pool(name="sbb", bufs=kb["sbb_bufs"]))
                    tmpp = p2.enter_context(tc.tile_pool(name="tmpp", bufs=kb["tmp_bufs"]))
                    yp = p2.enter_context(tc.tile_pool(name="yp", bufs=kb["y_bufs"]))
                    ymp = p2.enter_context(tc.tile_pool(name="ymp", bufs=1))
                    psSc = p2.enter_context(
                        tc.tile_pool(name="psSc", bufs=kb["psSc_bufs"], space="PSUM")
                    )
                    psPV = p2.enter_context(
                        tc.tile_pool(name="psPV", bufs=kb["psPV_bufs"], space="PSUM")
                    )
                    psBc = p2.enter_context(
                        tc.tile_pool(name="psBc", bufs=kb["psBc_bufs"], space="PSUM")
                    )

                    if kb["dma_order"]:
                        # wp/bproj are first needed by phase 2/3 — loading
                        # them here keeps the phase-1 critical DMAs in front.
                        nc.sync.dma_start(out=bproj_sb, in_=bproj[:, :])
                        nc.sync.dma_start(out=wp_sb[:, :, :], in_=wpT_r[:, :, :])

                    if kb.get("only_phase1"):
                        heads = []
                    else:
                        heads = list(range(H))

                    # V projection, second half: interleaved into the head loop
                    # (fills the PE gap while it waits for the sum reciprocal).
                    if heads and NVCH > 1:
                        wv2 = wvp.tile(
                            [P, CT, VCH], F16, tag="wv", bufs=1, name="wv2"
                        )
                        nc.sync.dma_start(out=wv2, in_=wvT_r[:, :, VCH : 2 * VCH])

                    def emit_scores(h):
                        par, ot = h % HPT, h // HPT
                        rows = slice(par * D, (par + 1) * D)
                        pt = ptp.tile([P, KT, LQ], BF16, tag="pt", name="pt")
                        for kt in range(KT):
                            ps_s = psSc.tile([P, LQ], F32, tag="ps_s", name="ps_s")
                            for ch in range(NCH):
                                sl = slice(ch * CH, (ch + 1) * CH)
                                nc.tensor.matmul(
                                    ps_s[:, sl],
                                    knT[rows, ot, kt * P : (kt + 1) * P],
                                    qnT[rows, ot, sl],
                                    start=True,
                                    stop=True,
                                )
                            nc.scalar.activation(pt[:, kt, :], ps_s, AF.Exp)
                        return pt

                    def emit_pv(h, pt):
                        rsum = rsp.tile([P, LQ], BF16, tag="rsum", name="rsum")
                        pvs = []
                        for ch in range(NCH):
                            sl = slice(ch * CH, (ch + 1) * CH)
                            pv = psPV.tile(
                                [2 * D, CH], F32, tag="ps_pv", name="ps_pv"
                            )
                            pvs.append(pv)
                            for kt in range(KT):
                                nc.tensor.matmul(
                                    pv,
                                    v_aug[:, kt, h, :],
                                    pt[:, kt, sl],
                                    start=(kt == 0),
                                    stop=(kt == KT - 1),
                                )
                            if kb["recip_fast"]:
                                # ~51-ULP custom-DVE reciprocal, ~5x faster
                                # than the iterative DVE reciprocal; bf16
                                # output cast happens on the DVE write port
                                from concourse.dve_ops import (
                                    RECIP_APPROX_FAST_CONSTS as _RC,
                                    RECIPROCAL_APPROX_FAST as _RF,
                                )
                                nc.vector._custom_dve(
                                    _RF,
                                    out=rsum[0:1, sl],
                                    in0=pv[0:1, :],
                                    s0=_RC["s0"], s1=_RC["s1"],
                                    imm2=_RC["imm2"],
                                )
                            else:
                                with nc.allow_low_precision(
                                    reason="bf16 softmax sum"
                                ):
                                    nc.vector.reciprocal(
                                        rsum[0:1, sl], pv[0:1, :]
                                    )
                        return pvs, rsum

                    def emit_tail(h, pvs, rsum):
                        par, ot = h % HPT, h // HPT
                        rows = slice(par * D, (par + 1) * D)
                        for ch in range(NCH):
                            sl = slice(ch * CH, (ch + 1) * CH)
                            ps_b = psBc.tile([D, CH], F32, tag="ps_b", name="ps_b")
                            nc.tensor.matmul(
                                ps_b,
                                ones64b[0:1, :],
                                rsum[0:1, sl],
                                start=True,
                                stop=True,
                            )
                            sb_b = sbb.tile([D, CH], F32, tag="sb_b", name="sb_b")
                            if kb["sb_evac"] == "gpsimd":
                                nc.gpsimd.tensor_copy(sb_b, ps_b)
                            else:
                                nc.vector.tensor_copy(sb_b, ps_b)
                            nc.vector.tensor_mul(
                                oT[rows, ot, sl], pvs[ch][D : 2 * D, :], sb_b
                            )

                    bias_bc = None
                    if heads and not (
                        bool(kb.get("split_out", 1))
                        and H >= 16
                        and not (bool(kb.get("pair", 1)) and HPT == 2)
                    ):
                        # broadcast bproj across partitions once; phase 3
                        # then folds the bias into the psum evac (tensor_add)
                        # instead of 2 extra matmuls per yt tile
                        ps_bb = psSc.tile([P, LQ], F32, tag="ps_s", name="ps_bb")
                        for vch in range(NVCH):
                            sl = slice(vch * VCH, (vch + 1) * VCH)
                            nc.tensor.matmul(
                                ps_bb[:, sl], ones1, bproj_sb[:, sl],
                                start=True, stop=True,
                            )
                        bias_bc = ymp.tile([P, C], BF16, name="bias_bc")
                        nc.vector.tensor_copy(bias_bc, ps_bb[:, 0:C])

                    use_pair = bool(kb.get("pair", 1)) and HPT == 2 and heads
                    # pair mode needs 4 pt buffers; drop y_mid to fit SBUF
                    split_out = (
                        bool(kb.get("split_out", 1)) and H >= 16 and not use_pair
                    )
                    ptb = 4 if use_pair else None
                    y_mid = None
                    if split_out:
                        y_mid = ymp.tile([P, LQ // P, C], BF16, name="y_mid")

                    def emit_out_half1(u):
                        # u indexes (yt, vch) units; contraction tiles ct<CT/2
                        yt, vch = divmod(u, NVCH)
                        sl = slice(vch * VCH, (vch + 1) * VCH)
                        ps_h = psPV.tile(
                            [P, VCH], F32, tag="ps_pv", name="ps_h"
                        )
                        for ct in range(CT // 2):
                            nc.tensor.matmul(
                                ps_h,
                                oT[:, ct, yt * P : (yt + 1) * P],
                                wp_sb[:, ct, sl],
                                start=(ct == 0),
                                stop=(ct == CT // 2 - 1),
                            )
                        nc.vector.tensor_copy(y_mid[:, yt, sl], ps_h)

                    def emit_scores_pair(h0, h1):
                        """Scores+exp for an even/odd head pair. The two
                        heads' matmuls are interleaved: they sit on PE row
                        groups 0-1 and 2-3 (base partitions 0 and 64), so
                        adjacent matmuls execute concurrently on hardware."""
                        ot = h0 // HPT
                        r0 = slice(0, D)
                        r1 = slice(D, 2 * D)
                        pt0 = ptp.tile(
                            [P, KT, LQ], BF16, tag="pt", name="pt0", bufs=ptb
                        )
                        pt1 = ptp.tile(
                            [P, KT, LQ], BF16, tag="pt", name="pt1", bufs=ptb
                        )
                        for kt in range(KT):
                            kl = slice(kt * P, (kt + 1) * P)
                            s0 = psSc.tile([P, LQ], F32, tag="ps_s", name="s0")
                            s1 = psSc.tile([P, LQ], F32, tag="ps_s", name="s1")
                            for ch in range(NCH):
                                sl = slice(ch * CH, (ch + 1) * CH)
                                nc.tensor.matmul(
                                    s0[:, sl], knT[r0, ot, kl],
                                    qnT[r0, ot, sl], start=True, stop=True,
                                )
                                nc.tensor.matmul(
                                    s1[:, sl], knT[r1, ot, kl],
                                    qnT[r1, ot, sl], start=True, stop=True,
                                )
                            nc.scalar.activation(pt0[:, kt, :], s0, AF.Exp)
                            nc.scalar.activation(pt1[:, kt, :], s1, AF.Exp)
                        return pt0, pt1

                    nunits = (LQ // P) * NVCH
                    emitted_units = 0
                    if use_pair:
                        def process_pair(pr, pts):
                            for i, hp in enumerate(pr):
                                pvs, rsum = emit_pv(hp, pts[i])
                                if NVCH > 1 and hp < KT:
                                    emit_vproj(1, hp, wv2, psPV, "ps_pv")
                                emit_tail(hp, pvs, rsum)

                        pend = None
                        for pi in range(len(heads) // 2):
                            pr = (heads[2 * pi], heads[2 * pi + 1])
                            pts = emit_scores_pair(*pr)
                            if pend is not None:
                                process_pair(*pend)
                            pend = (pr, pts)
                        if pend is not None:
                            process_pair(*pend)
                    else:
                        pend = None
                        for h in heads:
                            pt = emit_scores(h)
                            if pend is not None:
                                hp, ptp_ = pend
                                pvs, rsum = emit_pv(hp, ptp_)
                                if NVCH > 1 and hp < KT:
                                    emit_vproj(1, hp, wv2, psPV, "ps_pv")
                                emit_tail(hp, pvs, rsum)
                                if split_out and hp >= H - KT:
                                    u0 = (hp - (H - KT)) * 2
                                    for u in range(u0, min(u0 + 2, nunits)):
                                        emit_out_half1(u)
                                        emitted_units = max(
                                            emitted_units, u + 1
                                        )
                            pend = (h, pt)
                        if pend is not None:
                            hp, ptp_ = pend
                            pvs, rsum = emit_pv(hp, ptp_)
                            emit_tail(hp, pvs, rsum)
                    if split_out:
                        for u in range(emitted_units, nunits):
                            emit_out_half1(u)

                    # ============ PHASE 3: output projection ================
                    ct0 = CT // 2 if split_out else 0
                    for yt in ([] if kb.get("only_phase1") else range(LQ // P)):
                        ps_y = psSc.tile([P, C], F32, tag="ps_s", name="ps_y")
                        y_sb = yp.tile([P, C], F32, tag="y_sb", name="y_sb")
                        chunk_evac = (
                            bias_bc is not None and yt == LQ // P - 1
                        )
                        for vch in range(NVCH):
                            sl = slice(vch * VCH, (vch + 1) * VCH)
                            for ct in range(ct0, CT):
                                nc.tensor.matmul(
                                    ps_y[:, sl],
                                    oT[:, ct, yt * P : (yt + 1) * P],
                                    wp_sb[:, ct, sl],
                                    start=(ct == ct0),
                                    stop=(bias_bc is not None and ct == CT - 1),
                                )
                            if bias_bc is None:
                                nc.tensor.matmul(
                                    ps_y[:, sl],
                                    ones1,
                                    bproj_sb[:, sl],
                                    start=False,
                                    stop=True,
                                )
                            if chunk_evac:
                                # last tile: per-chunk evac+store right after
                                # the chunk's matmuls shortens the drain tail
                                nc.vector.tensor_add(
                                    y_sb[:, sl], ps_y[:, sl], bias_bc[:, sl]
                                )
                                nc.sync.dma_start(
                                    out=y_r[:, yt, sl], in_=y_sb[:, sl]
                                )
                        if not chunk_evac:
                            if split_out:
                                nc.vector.tensor_add(y_sb, ps_y, y_mid[:, yt, :])
                            elif bias_bc is not None:
                                nc.vector.tensor_add(y_sb, ps_y, bias_bc)
                            else:
                                nc.vector.tensor_copy(y_sb, ps_y)
                            nc.sync.dma_start(out=y_r[:, yt, :], in_=y_sb)

    nc.finalize()
    return nc


_NC_CACHE = {}


def _get_nc(C, H, LQ, LKV):
    key = (C, H, LQ, LKV)
    if key not in _NC_CACHE:
        _NC_CACHE[key] = build_nc(C, H, LQ, LKV)
    return _NC_CACHE[key]


def _host_inputs(q, kv, Wq, Wkv, Wproj, bproj, tau, H):
    B, LQ, C = q.shape
    LKV = kv.shape[1]
    P, D = 128, C // H
    HPT = P // D

    f16 = lambda a: np.ascontiguousarray(np.asarray(a, dtype=np.float32).astype(np.float16))
    bf16 = lambda a: np.ascontiguousarray(
        np.asarray(a, dtype=np.float32).astype(ml_dtypes.bfloat16)
    )

    wqT = f16(np.asarray(Wq).T)
    wkT = f16(np.asarray(Wkv)[:C].T)
    wvT = f16(np.asarray(Wkv)[C:].T)
    wpT = bf16(np.asarray(Wproj).T)
    bp = bf16(np.asarray(bproj).reshape(1, C))
    tau_b = np.full((P, 1), float(np.asarray(tau)), dtype=np.float32)
    tau2_b = np.full((P, 1), float(np.asarray(tau)) ** 2, dtype=np.float32)
    ones_blk = np.zeros((P, HPT), dtype=np.float16)
    for p in range(P):
        ones_blk[p, p // D] = 1.0
    blk2 = np.ascontiguousarray(ones_blk.T)

    shared = {
        "wqT": wqT, "wkT": wkT, "wvT": wvT, "wpT": wpT, "bproj": bp,
        "tau_b": tau_b, "tau2_b": tau2_b, "ones_blk": ones_blk, "blk2": blk2,
    }
    qn = np.asarray(q, dtype=np.float32)
    kvn = np.asarray(kv, dtype=np.float32)
    in_maps = []
    for b in range(B):
        m = dict(shared)
        m["qT"] = f16(qn[b].T)
        m["kvT"] = f16(kvn[b].T)
        in_maps.append(m)
    return in_maps


def kernel(q, kv, Wq, Wkv, Wproj, bproj, tau, _trace=False):
    B, LQ, C = q.shape
    LKV = kv.shape[1]
    H = 16 if C == 1024 else max(1, C // 64)
    assert B == NCORES, f"expected B == {NCORES}, got {B}"

    nc = _get_nc(C, H, LQ, LKV)
    in_maps = _host_inputs(q, kv, Wq, Wkv, Wproj, bproj, tau, H)
    res = run_bass_kernel_spmd(
        nc, in_maps, core_ids=list(range(NCORES)), trace=_trace
    )
    out = np.stack([res.results[b]["y"] for b in range(B)], axis=0)
    out = out.astype(np.asarray(q).dtype)
    if _trace:
        kernel._last_result = res
    return out



# revision 34
# speedup vs baseline: 1.0796x; 1.0022x over previous
"""CrossAttention (cosine-sim, learnable temperature) Trainium2 kernel.

Math (per batch element b, reference in fp32):
    qh  = (q @ Wq.T)   -> [Lq, C] -> heads [H, Lq, D]
    k,v = (kv @ Wkv.T) -> k,v [H, Lkv, D]
    qn = qh / (||qh||_d + eps); kn = k / (||k||_d + eps)
    attn = softmax(qn @ kn.T / tau); out = attn @ v
    y = out @ Wproj.T + bproj

Distribution: pure data-parallel over B=8 across the 8 NeuronCores (one
batch element per core, weights replicated, no collectives).

Device layout strategy: everything is kept "feature on partitions"
(transposed) so every matmul contraction dim lands on partitions:
    qT/kvT [C, L] (host pre-transposed), projections produce qnT/knT
    [C_out, L]; scores S^T [lkv, lq] = knT.T @ qnT per head; exp on ACT;
    P^T [lkv, lq] (bf16); out^T = [v | 1].T @ P^T gives both attn@v and
    the softmax sums (ones column); division by the sum and the final
    projection stay in the transposed domain; y [Lq, C] comes out in
    natural layout.

Normalization trick: rq = 1/(||qh||+eps) is applied to Q, and
rk/tau = 1/((||k||+eps)*tau) to K, before the scores matmul, so softmax
needs no further scaling.  Sum-of-squares over the head dim (on
partitions) is computed with a block-ones matmul; the per-row scales are
broadcast back across partitions with tiny K=2 / K=1 matmuls.
"""

import sys

sys.path.insert(0, "/opt/trn_rl_repo")

import numpy as np
import ml_dtypes

import concourse.bass as bass
import concourse.bacc as bacc
import concourse.mybir as mybir
from concourse.tile import TileContext
from concourse.bass_utils import run_bass_kernel_spmd

AF = mybir.ActivationFunctionType
ALU = mybir.AluOpType
F32 = mybir.dt.float32
F32R = mybir.dt.float32r
F16 = mybir.dt.float16
BF16 = mybir.dt.bfloat16

EPS = 1e-6
NCORES = 8


def r(ap):
    """fp32 AP -> float32r view (full-rate PE matmul on fp32 data)."""
    return ap.bitcast(F32R)


DEFAULT_KNOBS = dict(
    wt_bufs=2, sq_bufs=3, rbs_bufs=3, smalls_bufs=4,
    psA_bufs=4, psS_bufs=2, psB_bufs=2,
    pt_bufs=2, psSc_bufs=2, psPV_bufs=3, psBc_bufs=1,
    sbb_bufs=3, rsum_bufs=2, tmp_bufs=2, y_bufs=2,
    rb_evac="vector",         # rb broadcast psum -> SBUF evac engine
    sb_evac="vector",      # sb_b (1/sum broadcast) evac engine
    # NOTE: GPSIMD cannot access PSUM on TRN2 (BIR verifier) — psum
    # evacuations must go through ACT or DVE.
    norm_rsqrt=1,          # ACT Abs_reciprocal_sqrt for 1/(tau*||x||)
    recip_fast=1,          # custom-DVE reciprocal_approx_fast for 1/sum
    dma_order=1,           # priority-ordered input DMAs + per-ot W chunks
)


def build_nc(C=1024, H=16, LQ=1024, LKV=1024, knobs=None):
    kb = dict(DEFAULT_KNOBS)
    if knobs:
        kb.update(knobs)
    D = C // H          # head dim (64)
    P = 128
    OT = C // P         # feature tiles (8)
    CT = C // P         # contraction tiles (8)
    KT = LKV // P       # lkv partition tiles (8)
    HPT = P // D        # heads per 128-tile (2)
    CH = min(512, LQ)   # free-dim chunk per psum bank (fp32)
    NCH = LQ // CH      # chunks of Lq (2)
    VCH = min(512, C)   # chunk of output features for V projection
    NVCH = C // VCH

    nc = bacc.Bacc("TRN2", target_bir_lowering=False)

    qT = nc.dram_tensor("qT", [C, LQ], F16, kind="ExternalInput")
    kvT = nc.dram_tensor("kvT", [C, LKV], F16, kind="ExternalInput")
    wqT = nc.dram_tensor("wqT", [C, C], F16, kind="ExternalInput")
    wkT = nc.dram_tensor("wkT", [C, C], F16, kind="ExternalInput")
    wvT = nc.dram_tensor("wvT", [C, C], F16, kind="ExternalInput")
    wpT = nc.dram_tensor("wpT", [C, C], BF16, kind="ExternalInput")
    bproj = nc.dram_tensor("bproj", [1, C], BF16, kind="ExternalInput")
    tau_b = nc.dram_tensor("tau_b", [P, 1], F32, kind="ExternalInput")
    tau2_b = nc.dram_tensor("tau2_b", [P, 1], F32, kind="ExternalInput")
    ones_blk = nc.dram_tensor("ones_blk", [P, HPT], F16, kind="ExternalInput")
    blk2 = nc.dram_tensor("blk2", [HPT, P], F16, kind="ExternalInput")
    y = nc.dram_tensor("y", [LQ, C], F32, kind="ExternalOutput")

    qT_r = qT.rearrange("(ct p) l -> p ct l", p=P)
    kvT_r = kvT.rearrange("(ct p) l -> p ct l", p=P)
    wqT_r = wqT.rearrange("(ct p) o -> p ct o", p=P)
    wkT_r = wkT.rearrange("(ct p) o -> p ct o", p=P)
    wvT_r = wvT.rearrange("(ct p) o -> p ct o", p=P)
    wpT_r = wpT.rearrange("(ct p) o -> p ct o", p=P)
    y_r = y.rearrange("(yt p) o -> p yt o", p=P)

    with TileContext(nc) as tc:
        from contextlib import ExitStack

        with ExitStack() as stk:
            # ---------- persistent pools (live for the whole kernel) ----
            persist = stk.enter_context(tc.tile_pool(name="persist", bufs=1))
            qnT = persist.tile([P, OT, LQ], F16)      # qh * rq, transposed
            knT = persist.tile([P, OT, LKV], F16)     # k * rk/tau, transposed
            v_aug = persist.tile([P, KT, H, 2 * D], BF16)  # [ones|pad|v]
            oT = persist.tile([P, CT, LQ], BF16)       # (attn@v)/sum, transposed
            wp_sb = persist.tile([P, CT, C], BF16)
            consts = stk.enter_context(tc.tile_pool(name="consts", bufs=1))
            ones_blk_sb = consts.tile([P, HPT], F16)
            blk2_sb = consts.tile([HPT, P], F16)
            tau_sb = consts.tile([P, 1], F32)
            tau2_sb = consts.tile([P, 1], F32)
            ones64 = consts.tile([P, D], F32)
            ones64b = consts.tile([P, D], BF16)
            ones1 = consts.tile([1, P], BF16)
            bproj_sb = consts.tile([1, C], BF16)

            if not kb["dma_order"]:
                nc.sync.dma_start(out=ones_blk_sb, in_=ones_blk[:, :])
                nc.sync.dma_start(out=blk2_sb, in_=blk2[:, :])
                nc.sync.dma_start(out=tau_sb, in_=tau_b[:, :])
                nc.sync.dma_start(out=tau2_sb, in_=tau2_b[:, :])
            nc.vector.memset(ones64, 1.0)
            nc.vector.memset(ones64b, 1.0)
            nc.vector.memset(ones1, 1.0)
            # ones column FIRST: the softmax sum then lands on psum
            # partition 0, where the custom-DVE reciprocal is valid
            # (custom DVE ops silently no-op at partition offset != 0).
            nc.vector.memset(v_aug[:, :, :, 0:1], 1.0)
            nc.vector.memset(v_aug[:, :, :, 1:D], 0.0)
            if not kb["dma_order"]:
                nc.sync.dma_start(out=bproj_sb, in_=bproj[:, :])
                for ct in range(CT):
                    nc.sync.dma_start(out=wp_sb[:, ct, :], in_=wpT_r[:, ct, :])

            # ================= PHASE 1: projections =====================
            # (the whole body can be emitted `reps` times for benchmarking —
            # back-to-back repetitions in one NEFF isolate steady-state time)
            for _rep in range(kb.get("reps", 1)):
              with ExitStack() as repstk:
                # kvT + the V-projection weight stream live past phase 1 (the
                # second half of the V projection is interleaved into phase 2)
                p15 = repstk.enter_context(tc.tile_pool(name="p15", bufs=1))
                kvT_sb = p15.tile([P, CT, LKV], F16)
                wvp = repstk.enter_context(tc.tile_pool(name="wvp", bufs=1))

                if not kb["dma_order"]:
                    for ct in range(CT):
                        nc.sync.dma_start(out=kvT_sb[:, ct, :], in_=kvT_r[:, ct, :])

                hpc = VCH // D  # heads per v-projection chunk

                def emit_vproj(vch, vt, wv_t, pool, tag):
                    """One [128 lkv x VCH] tile of the V projection (natural
                    layout), written into the interleaved [v | ones] buffer."""
                    pv = pool.tile([P, VCH], F32, tag=tag)
                    for ct in range(CT):
                        nc.tensor.matmul(
                            pv,
                            kvT_sb[:, ct, vt * P : (vt + 1) * P],
                            wv_t[:, ct, :],
                            start=(ct == 0),
                            stop=(ct == CT - 1),
                        )
                    # ACT evac: phase-1 DVE is backlogged with norm muls —
                    # using ACT here unblocks phase-2's first PV LDWEIGHTS
                    nc.scalar.copy(
                        v_aug[:, vt, vch * hpc : (vch + 1) * hpc, D : 2 * D],
                        pv.rearrange("p (h d) -> p h d", d=D),
                    )

                with ExitStack() as p1:
                    ins = p1.enter_context(tc.tile_pool(name="ins", bufs=1))
                    qT_sb = ins.tile([P, CT, LQ], F16)
                    wst = p1.enter_context(tc.tile_pool(name="wst", bufs=2))
                    sqp = p1.enter_context(tc.tile_pool(name="sqp", bufs=kb["sq_bufs"]))
                    smalls = p1.enter_context(tc.tile_pool(name="smalls", bufs=kb["smalls_bufs"]))
                    rbs = p1.enter_context(tc.tile_pool(name="rbs", bufs=kb["rbs_bufs"]))
                    psA = p1.enter_context(
                        tc.tile_pool(name="psA", bufs=kb["psA_bufs"], space="PSUM")
                    )
                    psS = p1.enter_context(
                        tc.tile_pool(name="psS", bufs=kb["psS_bufs"], space="PSUM")
                    )
                    psB = p1.enter_context(
                        tc.tile_pool(name="psB", bufs=kb["psB_bufs"], space="PSUM")
                    )

                    wfull = {}
                    if kb["dma_order"]:
                        nc.sync.dma_start(
                            out=qT_sb[:, :, 0:CH], in_=qT_r[:, :, 0:CH]
                        )
                    else:
                        for ct in range(CT):
                            nc.sync.dma_start(
                                out=qT_sb[:, ct, :], in_=qT_r[:, ct, :]
                            )
                    if kb["dma_order"] and kb.get("wfull", 1):
                        # priority order on the in-order DMA queue: wq in
                        # per-ot chunks (first matmul waits only chunk 0),
                        # then kvT, then wk chunks.
                        for side in range(2):
                            w_r = wqT_r if side == 0 else wkT_r
                            wf = wst.tile(
                                [P, CT, C], F16, tag=f"wf{side}", bufs=1,
                                name="wf",
                            )
                            wfull[side] = wf
                            for ot in range(OT):
                                sl = slice(ot * P, (ot + 1) * P)
                                nc.sync.dma_start(
                                    out=wf[:, :, sl], in_=w_r[:, :, sl]
                                )
                                if side == 0 and ot == 0:
                                    nc.sync.dma_start(
                                        out=qT_sb[:, :, CH:LQ],
                                        in_=qT_r[:, :, CH:LQ],
                                    )
                                    nc.sync.dma_start(
                                        out=ones_blk_sb, in_=ones_blk[:, :]
                                    )
                                    nc.sync.dma_start(
                                        out=blk2_sb, in_=blk2[:, :]
                                    )
                                    nc.sync.dma_start(
                                        out=tau_sb, in_=tau_b[:, :]
                                    )
                                    nc.sync.dma_start(
                                        out=tau2_sb, in_=tau2_b[:, :]
                                    )
                            if side == 0:
                                nc.sync.dma_start(
                                    out=kvT_sb[:, :, :], in_=kvT_r[:, :, :]
                                )

                    # --- software-pipelined projection+norm chunks ------------
                    # stage A (emit_mm):  proj matmuls -> ph psum; Square -> sq
                    # stage B (emit_ssq): block-ones matmul -> ssq; sqrt; +eps;
                    #                     reciprocal -> rrs
                    # stage C (emit_tail): broadcast matmul -> rb; evac; multiply
                    #                     -> qnT/knT (releases ph)
                    # Emission order interleaves stages two chunks apart so the
                    # in-order PE stream never waits on an ACT/DVE round-trip.
                    class Job:
                        pass

                    def stage_A(j):
                        j.ph = psA.tile([P, CH], F32, tag="ph", name="ph")
                        for ct in range(CT):
                            nc.tensor.matmul(
                                j.ph,
                                j.wt[:, ct, :],
                                j.x_sb[:, ct, j.sl],
                                start=(ct == 0),
                                stop=(ct == CT - 1),
                            )
                        j.sq = sqp.tile([P, CH], F16, tag="sq", name="sq")
                        nc.scalar.activation(j.sq, j.ph, AF.Square)

                    def stage_B(j):
                        j.ssq = psS.tile([HPT, CH], F32, tag="ssq", name="ssq")
                        nc.tensor.matmul(
                            j.ssq, ones_blk_sb, j.sq, start=True, stop=True
                        )
                        j.rrs = smalls.tile([HPT, CH], F16, tag="rrs", name="rrs")
                        if kb["norm_rsqrt"]:
                            # 1/(tau*||x||) = rsqrt(ssq * tau^2), single ACT op
                            # (eps dropped: ||x|| ~ 8 >> eps)
                            nc.scalar.activation(
                                j.rrs, j.ssq, AF.Abs_reciprocal_sqrt,
                                scale=tau2_sb[:HPT, :] if j.with_tau else 1.0,
                            )
                            return
                        rr = smalls.tile([HPT, CH], F32, tag="rr", name="rr")
                        nc.scalar.activation(rr, j.ssq, AF.Sqrt)
                        if j.with_tau:
                            nc.vector.tensor_scalar(
                                rr, rr, EPS, tau_sb[:HPT, :], op0=ALU.add,
                                op1=ALU.mult,
                            )
                        else:
                            nc.vector.tensor_scalar_add(rr, rr, EPS)
                        with nc.allow_low_precision(reason="fp16 inverse scale"):
                            nc.vector.reciprocal(j.rrs, rr)

                    def stage_C(j):
                        rb = psB.tile([P, CH], F32, tag="rb", name="rb")
                        nc.tensor.matmul(rb, blk2_sb, j.rrs, start=True, stop=True)
                        rb_sb = rbs.tile([P, CH], F32, tag="rb_sb", name="rb_sb")
                        if kb["rb_evac"] == "act":
                            nc.scalar.copy(rb_sb, rb)
                        elif kb["rb_evac"] == "gpsimd":
                            nc.gpsimd.tensor_copy(rb_sb, rb)
                        else:
                            nc.vector.tensor_copy(rb_sb, rb)
                        nc.vector.tensor_mul(j.out_t[:, j.ot, j.sl], j.ph, rb_sb)

                    jobs = []
                    if not kb["dma_order"] and kb.get("wfull", 1):
                        for side in range(2):
                            w_r = wqT_r if side == 0 else wkT_r
                            wf = wst.tile(
                                [P, CT, C], F16, tag=f"wf{side}", bufs=1,
                                name="wf",
                            )
                            for ct in range(CT):
                                nc.sync.dma_start(
                                    out=wf[:, ct, :], in_=w_r[:, ct, :]
                                )
                            wfull[side] = wf
                    if kb["dma_order"]:
                        # q-side jobs first: their inputs land first on the
                        # in-order DMA queue.
                        ot_side = [(ot, 0) for ot in range(OT)] + [
                            (ot, 1) for ot in range(OT)
                        ]
                    else:
                        ot_side = [
                            (ot, side) for ot in range(OT) for side in range(2)
                        ]
                    for ot, side in ot_side:
                            w_r = wqT_r if side == 0 else wkT_r
                            x_sb = qT_sb if side == 0 else kvT_sb
                            out_t = qnT if side == 0 else knT
                            L = LQ if side == 0 else LKV
                            if kb.get("wfull", 1):
                                wt = wfull[side][:, :, ot * P : (ot + 1) * P]
                            else:
                                wt = wst.tile(
                                    [P, CT, P], F16, tag="wt",
                                    bufs=kb["wt_bufs"], name="wt",
                                )
                                nc.sync.dma_start(
                                    out=wt,
                                    in_=w_r[:, :, ot * P : (ot + 1) * P],
                                )
                            for ch in range(L // CH):
                                j = Job()
                                j.wt, j.x_sb, j.out_t = wt, x_sb, out_t
                                j.ot, j.sl = ot, slice(ch * CH, (ch + 1) * CH)
                                j.with_tau = side == 1
                                jobs.append(j)

                    pd_b = kb.get("pd_b", 1)
                    pd_c = kb.get("pd_c", 2)
                    if kb.get("skip_square"):
                        def stage_A(j, _A=stage_A):
                            j.ph = psA.tile([P, CH], F32, tag="ph", name="ph")
                            for ct in range(CT):
                                nc.tensor.matmul(
                                    j.ph, j.wt[:, ct, :], j.x_sb[:, ct, j.sl],
                                    start=(ct == 0), stop=(ct == CT - 1),
                                )
                            j.sq = None
                        nc.vector.tensor_copy(qnT[:, 0, 0:CH], jobs[0].wt[:, 0, :].bitcast(F16)) if False else None
                    if kb.get("skip_tails"):
                        def stage_B(j):
                            pass
                        if kb.get("skip_evac"):
                            def stage_C(j):
                                pass
                        else:
                            def stage_C(j):
                                nc.scalar.activation(
                                    j.out_t[:, j.ot, j.sl], j.ph, AF.Copy
                                )
                    for i, j in enumerate(jobs):
                        stage_A(j)
                        if i >= pd_b:
                            stage_B(jobs[i - pd_b])
                        if i >= pd_c:
                            stage_C(jobs[i - pd_c])
                    for i in range(len(jobs) - pd_b, len(jobs)):
                        stage_B(jobs[i])
                        if i - pd_c + pd_b >= 0 and i - pd_c + pd_b < len(jobs) and i - pd_c + pd_b >= len(jobs) - pd_c:
                            pass
                    for i in range(len(jobs) - pd_c, len(jobs)):
                        stage_C(jobs[i])

                    # first half of the V projection (heads 0..hpc-1)
                    if not kb.get("skip_vproj0"):
                        wv_t = wvp.tile([P, CT, VCH], F16, tag="wv", bufs=1, name="wv")
                        nc.sync.dma_start(out=wv_t, in_=wvT_r[:, :, 0:VCH])
                        for vt in range(KT):
                            emit_vproj(0, vt, wv_t, psA, "ph")

                # ================= PHASE 2: attention per head ==============
                with ExitStack() as p2:
                    ptp = p2.enter_context(tc.tile_pool(name="ptp", bufs=kb["pt_bufs"]))
                    rsp = p2.enter_context(tc.tile_pool(name="rsp", bufs=kb["rsum_bufs"]))
                    sbb = p2.enter_context(tc.tile_pool(name="sbb", bufs=kb["sbb_bufs"]))
                    tmpp = p2.enter_context(tc.tile_pool(name="tmpp", bufs=kb["tmp_bufs"]))
                    yp = p2.enter_context(tc.tile_pool(name="yp", bufs=kb["y_bufs"]))
                    ymp = p2.enter_context(tc.tile_pool(name="ymp", bufs=1))
                    psSc = p2.enter_context(
                        tc.tile_pool(name="psSc", bufs=kb["psSc_bufs"], space="PSUM")
                    )
                    psPV = p2.enter_context(
                        tc.tile_pool(name="psPV", bufs=kb["psPV_bufs"], space="PSUM")
                    )
                    psBc = p2.enter_context(
                        tc.tile_pool(name="psBc", bufs=kb["psBc_bufs"], space="PSUM")
                    )

                    if kb["dma_order"]:
                        # wp/bproj are first needed by phase 2/3 — loading
                        # them here keeps the phase-1 critical DMAs in front.
                        nc.sync.dma_start(out=bproj_sb, in_=bproj[:, :])
                        nc.sync.dma_start(out=wp_sb[:, :, :], in_=wpT_r[:, :, :])

                    if kb.get("only_phase1"):
                        heads = []
                    else:
                        heads = list(range(H))

                    # V projection, second half: interleaved into the head loop
                    # (fills the PE gap while it waits for the sum reciprocal).
                    if heads and NVCH > 1:
                        wv2 = wvp.tile(
                            [P, CT, VCH], F16, tag="wv", bufs=1, name="wv2"
                        )
                        nc.sync.dma_start(out=wv2, in_=wvT_r[:, :, VCH : 2 * VCH])

                    def emit_scores(h):
                        par, ot = h % HPT, h // HPT
                        rows = slice(par * D, (par + 1) * D)
                        pt = ptp.tile([P, KT, LQ], BF16, tag="pt", name="pt")
                        for kt in range(KT):
                            ps_s = psSc.tile([P, LQ], F32, tag="ps_s", name="ps_s")
                            for ch in range(NCH):
                                sl = slice(ch * CH, (ch + 1) * CH)
                                nc.tensor.matmul(
                                    ps_s[:, sl],
                                    knT[rows, ot, kt * P : (kt + 1) * P],
                                    qnT[rows, ot, sl],
                                    start=True,
                                    stop=True,
                                )
                            nc.scalar.activation(pt[:, kt, :], ps_s, AF.Exp)
                        return pt

                    def emit_pv(h, pt):
                        rsum = rsp.tile([P, LQ], BF16, tag="rsum", name="rsum")
                        pvs = []
                        for ch in range(NCH):
                            sl = slice(ch * CH, (ch + 1) * CH)
                            pv = psPV.tile(
                                [2 * D, CH], F32, tag="ps_pv", name="ps_pv"
                            )
                            pvs.append(pv)
                            for kt in range(KT):
                                nc.tensor.matmul(
                                    pv,
                                    v_aug[:, kt, h, :],
                                    pt[:, kt, sl],
                                    start=(kt == 0),
                                    stop=(kt == KT - 1),
                                )
                            if kb["recip_fast"]:
                                # ~51-ULP custom-DVE reciprocal, ~5x faster
                                # than the iterative DVE reciprocal; bf16
                                # output cast happens on the DVE write port
                                from concourse.dve_ops import (
                                    RECIP_APPROX_FAST_CONSTS as _RC,
                                    RECIPROCAL_APPROX_FAST as _RF,
                                )
                                nc.vector._custom_dve(
                                    _RF,
                                    out=rsum[0:1, sl],
                                    in0=pv[0:1, :],
                                    s0=_RC["s0"], s1=_RC["s1"],
                                    imm2=_RC["imm2"],
                                )
                            else:
                                with nc.allow_low_precision(
                                    reason="bf16 softmax sum"
                                ):
                                    nc.vector.reciprocal(
                                        rsum[0:1, sl], pv[0:1, :]
                                    )
                        return pvs, rsum

                    def emit_tail(h, pvs, rsum):
                        par, ot = h % HPT, h // HPT
                        rows = slice(par * D, (par + 1) * D)
                        for ch in range(NCH):
                            sl = slice(ch * CH, (ch + 1) * CH)
                            ps_b = psBc.tile([D, CH], F32, tag="ps_b", name="ps_b")
                            nc.tensor.matmul(
                                ps_b,
                                ones64b[0:1, :],
                                rsum[0:1, sl],
                                start=True,
                                stop=True,
                            )
                            sb_b = sbb.tile([D, CH], F32, tag="sb_b", name="sb_b")
                            if kb["sb_evac"] == "gpsimd":
                                nc.gpsimd.tensor_copy(sb_b, ps_b)
                            else:
                                nc.vector.tensor_copy(sb_b, ps_b)
                            nc.vector.tensor_mul(
                                oT[rows, ot, sl], pvs[ch][D : 2 * D, :], sb_b
                            )

                    bias_bc = None
                    if heads and not (
                        bool(kb.get("split_out", 1))
                        and H >= 16
                        and not (bool(kb.get("pair", 1)) and HPT == 2)
                    ):
                        # broadcast bproj across partitions once; phase 3
                        # then folds the bias into the psum evac (tensor_add)
                        # instead of 2 extra matmuls per yt tile
                        ps_bb = psSc.tile([P, LQ], F32, tag="ps_s", name="ps_bb")
                        for vch in range(NVCH):
                            sl = slice(vch * VCH, (vch + 1) * VCH)
                            nc.tensor.matmul(
                                ps_bb[:, sl], ones1, bproj_sb[:, sl],
                                start=True, stop=True,
                            )
                        bias_bc = ymp.tile([P, C], BF16, name="bias_bc")
                        nc.vector.tensor_copy(bias_bc, ps_bb[:, 0:C])

                    use_pair = bool(kb.get("pair", 1)) and HPT == 2 and heads
                    # pair mode needs 4 pt buffers; drop y_mid to fit SBUF
                    split_out = (
                        bool(kb.get("split_out", 1)) and H >= 16 and not use_pair
                    )
                    ptb = 4 if use_pair else None
                    y_mid = None
                    if split_out:
                        y_mid = ymp.tile([P, LQ // P, C], BF16, name="y_mid")

                    def emit_out_half1(u):
                        # u indexes (yt, vch) units; contraction tiles ct<CT/2
                        yt, vch = divmod(u, NVCH)
                        sl = slice(vch * VCH, (vch + 1) * VCH)
                        ps_h = psPV.tile(
                            [P, VCH], F32, tag="ps_pv", name="ps_h"
                        )
                        for ct in range(CT // 2):
                            nc.tensor.matmul(
                                ps_h,
                                oT[:, ct, yt * P : (yt + 1) * P],
                                wp_sb[:, ct, sl],
                                start=(ct == 0),
                                stop=(ct == CT // 2 - 1),
                            )
                        nc.vector.tensor_copy(y_mid[:, yt, sl], ps_h)

                    def emit_scores_pair(h0, h1):
                        """Scores+exp for an even/odd head pair. The two
                        heads' matmuls are interleaved: they sit on PE row
                        groups 0-1 and 2-3 (base partitions 0 and 64), so
                        adjacent matmuls execute concurrently on hardware."""
                        ot = h0 // HPT
                        r0 = slice(0, D)
                        r1 = slice(D, 2 * D)
                        pt0 = ptp.tile(
                            [P, KT, LQ], BF16, tag="pt", name="pt0", bufs=ptb
                        )
                        pt1 = ptp.tile(
                            [P, KT, LQ], BF16, tag="pt", name="pt1", bufs=ptb
                        )
                        for kt in range(KT):
                            kl = slice(kt * P, (kt + 1) * P)
                            s0 = psSc.tile([P, LQ], F32, tag="ps_s", name="s0")
                            s1 = psSc.tile([P, LQ], F32, tag="ps_s", name="s1")
                            for ch in range(NCH):
                                sl = slice(ch * CH, (ch + 1) * CH)
                                nc.tensor.matmul(
                                    s0[:, sl], knT[r0, ot, kl],
                                    qnT[r0, ot, sl], start=True, stop=True,
                                )
                                nc.tensor.matmul(
                                    s1[:, sl], knT[r1, ot, kl],
                                    qnT[r1, ot, sl], start=True, stop=True,
                                )
                            nc.scalar.activation(pt0[:, kt, :], s0, AF.Exp)
                            nc.scalar.activation(pt1[:, kt, :], s1, AF.Exp)
                        return pt0, pt1

                    nunits = (LQ // P) * NVCH
                    emitted_units = 0
                    if use_pair:
                        def process_pair(pr, pts):
                            for i, hp in enumerate(pr):
                                pvs, rsum = emit_pv(hp, pts[i])
                                if NVCH > 1 and hp < KT:
                                    emit_vproj(1, hp, wv2, psPV, "ps_pv")
                                emit_tail(hp, pvs, rsum)

                        pend = None
                        for pi in range(len(heads) // 2):
                            pr = (heads[2 * pi], heads[2 * pi + 1])
                            pts = emit_scores_pair(*pr)
                            if pend is not None:
                                process_pair(*pend)
                            pend = (pr, pts)
                        if pend is not None:
                            process_pair(*pend)
                    else:
                        pend = None
                        for h in heads:
                            pt = emit_scores(h)
                            if pend is not None:
                                hp, ptp_ = pend
                                pvs, rsum = emit_pv(hp, ptp_)
                                if NVCH > 1 and hp < KT:
                                    emit_vproj(1, hp, wv2, psPV, "ps_pv")
                                emit_tail(hp, pvs, rsum)
                                if split_out and hp >= H - KT:
                                    u0 = (hp - (H - KT)) * 2
                                    for u in range(u0, min(u0 + 2, nunits)):
                                        emit_out_half1(u)
                                        emitted_units = max(
                                            emitted_units, u + 1
                                        )
                            pend = (h, pt)
                        if pend is not None:
                            hp, ptp_ = pend
                            pvs, rsum = emit_pv(hp, ptp_)
                            emit_tail(hp, pvs, rsum)
                    if split_out:
                        for u in range(emitted_units, nunits):
                            emit_out_half1(u)

                    # ============ PHASE 3: output projection ================
                    ct0 = CT // 2 if split_out else 0
                    for yt in ([] if kb.get("only_phase1") else range(LQ // P)):
                        ps_y = psSc.tile([P, C], F32, tag="ps_s", name="ps_y")
                        y_sb = yp.tile([P, C], F32, tag="y_sb", name="y_sb")
                        chunk_evac = (
                            bias_bc is not None and yt == LQ // P - 1
                        )
                        for vch in range(NVCH):
                            sl = slice(vch * VCH, (vch + 1) * VCH)
                            for ct in range(ct0, CT):
                                nc.tensor.matmul(
                                    ps_y[:, sl],
                                    oT[:, ct, yt * P : (yt + 1) * P],
                                    wp_sb[:, ct, sl],
                                    start=(ct == ct0),
                                    stop=(bias_bc is not None and ct == CT - 1),
                                )
                            if bias_bc is None:
                                nc.tensor.matmul(
                                    ps_y[:, sl],
                                    ones1,
                                    bproj_sb[:, sl],
                                    start=False,
                                    stop=True,
                                )
                            if chunk_evac:
                                # last tile: per-chunk evac+store right after
                                # the chunk's matmuls shortens the drain tail
                                nc.vector.tensor_add(
                                    y_sb[:, sl], ps_y[:, sl], bias_bc[:, sl]
                                )
                                nc.sync.dma_start(
                                    out=y_r[:, yt, sl], in_=y_sb[:, sl]
                                )
                        if not chunk_evac:
                            if split_out:
                                nc.vector.tensor_add(y_sb, ps_y, y_mid[:, yt, :])
                            elif bias_bc is not None:
                                nc.vector.tensor_add(y_sb, ps_y, bias_bc)
                            else:
                                nc.vector.tensor_copy(y_sb, ps_y)
                            nc.sync.dma_start(out=y_r[:, yt, :], in_=y_sb)

    nc.finalize()
    return nc


_NC_CACHE = {}


def _get_nc(C, H, LQ, LKV):
    key = (C, H, LQ, LKV)
    if key not in _NC_CACHE:
        _NC_CACHE[key] = build_nc(C, H, LQ, LKV)
    return _NC_CACHE[key]


def _host_inputs(q, kv, Wq, Wkv, Wproj, bproj, tau, H):
    B, LQ, C = q.shape
    LKV = kv.shape[1]
    P, D = 128, C // H
    HPT = P // D

    f16 = lambda a: np.ascontiguousarray(np.asarray(a, dtype=np.float32).astype(np.float16))
    bf16 = lambda a: np.ascontiguousarray(
        np.asarray(a, dtype=np.float32).astype(ml_dtypes.bfloat16)
    )

    wqT = f16(np.asarray(Wq).T)
    wkT = f16(np.asarray(Wkv)[:C].T)
    wvT = f16(np.asarray(Wkv)[C:].T)
    wpT = bf16(np.asarray(Wproj).T)
    bp = bf16(np.asarray(bproj).reshape(1, C))
    tau_b = np.full((P, 1), float(np.asarray(tau)), dtype=np.float32)
    tau2_b = np.full((P, 1), float(np.asarray(tau)) ** 2, dtype=np.float32)
    ones_blk = np.zeros((P, HPT), dtype=np.float16)
    for p in range(P):
        ones_blk[p, p // D] = 1.0
    blk2 = np.ascontiguousarray(ones_blk.T)

    shared = {
        "wqT": wqT, "wkT": wkT, "wvT": wvT, "wpT": wpT, "bproj": bp,
        "tau_b": tau_b, "tau2_b": tau2_b, "ones_blk": ones_blk, "blk2": blk2,
    }
    qn = np.asarray(q, dtype=np.float32)
    kvn = np.asarray(kv, dtype=np.float32)
    in_maps = []
    for b in range(B):
        m = dict(shared)
        m["qT"] = f16(qn[b].T)
        m["kvT"] = f16(kvn[b].T)
        in_maps.append(m)
    return in_maps


def kernel(q, kv, Wq, Wkv, Wproj, bproj, tau, _trace=False):
    B, LQ, C = q.shape
    LKV = kv.shape[1]
    H = 16 if C == 1024 else max(1, C // 64)
    assert B == NCORES, f"expected B == {NCORES}, got {B}"

    nc = _get_nc(C, H, LQ, LKV)
    in_maps = _host_inputs(q, kv, Wq, Wkv, Wproj, bproj, tau, H)
    res = run_bass_kernel_spmd(
        nc, in_maps, core_ids=list(range(NCORES)), trace=_trace
    )
    out = np.stack([res.results[b]["y"] for b in range(B)], axis=0)
    out = out.astype(np.asarray(q).dtype)
    if _trace:
        kernel._last_result = res
    return out

